# revision 8
# baseline (speedup 1.0000x reference)
"""Trainium2 Bass kernel for masked multi-head graph attention (Velickovic).

Problem: h[B=4,G=2048,D=128], mask[G,G] (1=masked), W_Q/W_K[H=8,D,16], W_V[H,D,16]
  q,k,v = h @ W_*  per head; compat = 0.25 * q k^T; masked -> -inf;
  attn = softmax(compat); attn = where(mask, 0, attn); out = attn @ v.

Sharding: 8 cores; core c owns query rows [c*256, (c+1)*256) for ALL (b,h)
pairs. mask rows and h query rows are passed pre-sliced per core; k/v inputs
(full h) are replicated. No cross-device communication.

Per-core algorithm (fp16 on SBUF, fp32 PSUM for matmul accumulation):
  hT = h^T (PE transpose, fp16)      [D=128, G] per b
  kT[h'] = W_K^T hT  (PE, stationary cols padded to 32-row strips)
  qT[h'] = W_Q^T hqT
  v      = hT^T W_V                  [g, (h,v)] 32-wide padded slots:
                                     16 v cols | ones col | 15 zero cols
  attendT = transpose(mask==0) fp16 multiplier  [g, q]
  per (b,h): compatT[g,q] = kT^T qT (PE, 16-contract at row strip 32*h')
             e  = exp(0.25*compatT - 9)   (ACT, PSUM->SBUF fp16; the -9
                  shift cancels in softmax and keeps e within fp16 range)
             me = e * attendT             (DVE fp16 2x)
             S[(strip,v),q] += v_slot^T me  (PE col-tiled, 4 strips)
             out[q,:] = (S^T @ collapse)[:, :16] / (S^T @ collapse)[:, 16]

Hardware constraint honored throughout: a Matmult instruction can carry at
most ONE semaphore wait (walrus S3_LW limit). Every matmul SBUF operand is
produced by the DVE (so operand waits collapse into the single DVE sem),
every PSUM tile's reader in the setup phase is a DVE copy, and a strict
all-engine barrier separates setup from the attention loop so no stale
cross-engine waits surface on matmuls afterwards.
"""

import os
import sys

for p in ("/opt/trn_rl_repo", "/root/.axon_site", "/root/.axon_site/_ro/trn_rl_repo"):
    if os.path.isdir(p) and p not in sys.path:
        sys.path.append(p)

import numpy as np

import concourse.bacc as bacc
import concourse.bass as bass
import concourse.tile as tile
from concourse import mybir

B, G, D, H, K, V = 4, 2048, 128, 8, 16, 16
NCORES = 8
QS = G // NCORES  # 256 q rows per core
NJ = G // 128  # 16 g-blocks
EXP_BIAS = -9.0

f32 = mybir.dt.float32
f16 = mybir.dt.float16
i32 = mybir.dt.int32


def build_program():
    nc = bacc.Bacc("TRN2", target_bir_lowering=False, debug=False)

    h_in = nc.declare_dram_parameter("h", [B, G, D], f32, isOutput=False)
    hq_in = nc.declare_dram_parameter("hq", [B, QS, D], f32, isOutput=False)
    maskq_in = nc.declare_dram_parameter("maskq", [QS, G], i32, isOutput=False)
    wq_in = nc.declare_dram_parameter("W_Q", [H, D, K], f32, isOutput=False)
    wk_in = nc.declare_dram_parameter("W_K", [H, D, K], f32, isOutput=False)
    wv_in = nc.declare_dram_parameter("W_V", [H, D, V], f32, isOutput=False)
    ident_in = nc.declare_dram_parameter("ident", [128, 128], f16, isOutput=False)
    coll_in = nc.declare_dram_parameter("collapse", [128, 17], f32, isOutput=False)
    out_dram = nc.declare_dram_parameter("out", [B, H, QS, V], f32, isOutput=True)

    dma = nc.default_dma_engine

    with tile.TileContext(nc) as tc:
        with (
            tc.tile_pool(name="consts", bufs=1) as consts,
            tc.tile_pool(name="persist", bufs=1) as persist,
            tc.tile_pool(name="stage", bufs=4) as stage,
            tc.tile_pool(name="stagem", bufs=2) as stagem,
            tc.tile_pool(name="epool", bufs=4) as epool,
            tc.tile_pool(name="mepool", bufs=4) as mepool,
            tc.tile_pool(name="spool", bufs=4) as spool,
            tc.tile_pool(name="fpool", bufs=8) as fpool,
        ):
            # ---------------- constants (all DVE-laundered) ----------------
            ident_raw = consts.tile([128, 128], f16, tag="ident_raw")
            dma.dma_start(out=ident_raw[:, :], in_=ident_in[:])
            ident = consts.tile([128, 128], f16, tag="ident")
            nc.vector.tensor_copy(ident[:, :], ident_raw[:, :])

            coll_raw = consts.tile([128, 17], f32, tag="coll_raw")
            dma.dma_start(out=coll_raw[:, :], in_=coll_in[:])
            collapse = consts.tile([128, 17], f32, tag="collapse")
            nc.vector.tensor_copy(collapse[:, :], coll_raw[:, :])

            bias_sb = consts.tile([128, 1], f32, tag="bias")
            nc.vector.memset(bias_sb[:, :], EXP_BIAS)

            # ---------------- persistent SBUF slabs ----------------
            hT_sb = persist.tile([128, B * G], f16, tag="hT")  # [d, b*G+g]
            hqT_sb = persist.tile([128, B * QS], f16, tag="hqT")  # [d, b*QS+q]
            kT_sb = persist.tile([128, B * 2 * G], f16, tag="kT")
            qT_sb = persist.tile([128, B * 2 * QS], f16, tag="qT")
            v_sb = persist.tile([128, B * NJ * H * 32], f16, tag="v")
            att_sb = persist.tile([128, NJ * QS], f16, tag="att")

            wk4 = [
                consts.tile([128, 128], f16, tag=f"wk4_{q}", name=f"wk4_{q}")
                for q in range(2)
            ]
            wq4 = [
                consts.tile([128, 128], f16, tag=f"wq4_{q}", name=f"wq4_{q}")
                for q in range(2)
            ]
            wv_all = consts.tile([128, 128], f16, tag="wv_all")  # [d, 16h+v]

            with tc.tile_pool(name="ps_setup", bufs=2, space="PSUM") as ps_setup:
                # ---- W load + cast (stationary cols 32h'+0:16 = head 4q+h') ----
                for w_in, slabs in ((wk_in, wk4), (wq_in, wq4)):
                    wst = stage.tile([128, 128], f32, tag="wstage")
                    dma.dma_start(
                        out=wst[:, :].rearrange("d (h k) -> d h k", h=H),
                        in_=w_in[:].rearrange("h d k -> d h k"),
                    )
                    for q in range(2):
                        nc.vector.memset(slabs[q][:, :], 0.0)
                        dst = slabs[q][:, :].rearrange("p (s k) -> p s k", k=32)
                        nc.vector.tensor_copy(
                            dst[:, :, 0:16],
                            wst[:, 64 * q : 64 * (q + 1)].rearrange(
                                "p (s k) -> p s k", k=16
                            ),
                        )
                wst = stage.tile([128, 128], f32, tag="wstage")
                dma.dma_start(
                    out=wst[:, :].rearrange("d (h v) -> d h v", h=H),
                    in_=wv_in[:].rearrange("h d v -> d h v"),
                )
                nc.vector.tensor_copy(wv_all[:, :], wst[:, :])

                # ---- hT via fp16 PE transpose ----
                for b in range(B):
                    ps = ps_setup.tile([128, 2048], f16, tag="ps")
                    for j in range(NJ):
                        hst = stage.tile([128, 128], f32, tag="hstage")
                        dma.dma_start(
                            out=hst[:, :], in_=h_in[b, 128 * j : 128 * (j + 1), :]
                        )
                        h16 = stage.tile([128, 128], f16, tag="h16")
                        nc.vector.tensor_copy(h16[:, :], hst[:, :])
                        nc.tensor.matmul(
                            ps[:, 128 * j : 128 * (j + 1)],
                            lhsT=h16[:, :],
                            rhs=ident[:, :],
                            is_transpose=True,
                            start=(j % 8 == 0),
                            stop=(j % 8 == 7),
                        )
                    nc.vector.tensor_copy(hT_sb[:, G * b : G * (b + 1)], ps[:, :])

                ps = ps_setup.tile([128, 1024], f16, tag="ps")
                for b in range(B):
                    for j in range(2):
                        hst = stage.tile([128, 128], f32, tag="hstage")
                        dma.dma_start(
                            out=hst[:, :], in_=hq_in[b, 128 * j : 128 * (j + 1), :]
                        )
                        h16 = stage.tile([128, 128], f16, tag="h16")
                        nc.vector.tensor_copy(h16[:, :], hst[:, :])
                        jj = 2 * b + j
                        nc.tensor.matmul(
                            ps[:, 128 * jj : 128 * (jj + 1)],
                            lhsT=h16[:, :],
                            rhs=ident[:, :],
                            is_transpose=True,
                            start=(jj == 0),
                            stop=(jj == 7),
                        )
                nc.vector.tensor_copy(hqT_sb[:, :], ps[:, :])

                # ---- attendT = transpose(maskq == 0), fp16 ----
                psA = ps_setup.tile([128, 2048], f16, tag="ps", name="psA")
                psB = ps_setup.tile([128, 2048], f16, tag="ps", name="psB")
                for qh in range(2):
                    mi = stagem.tile([128, 2048], i32, tag="mi")
                    dma.dma_start(
                        out=mi[:, :], in_=maskq_in[128 * qh : 128 * (qh + 1), :]
                    )
                    att_f = stagem.tile([128, 2048], f16, tag="attf")
                    nc.vector.tensor_scalar(
                        att_f[:, :], mi[:, :], 0, None, op0=mybir.AluOpType.is_equal
                    )
                    for j in range(NJ):
                        ps = psA if j < 8 else psB
                        j8 = j % 8
                        off = j8 * 256 + qh * 128
                        nc.tensor.matmul(
                            ps[:, off : off + 128],
                            lhsT=att_f[:, 128 * j : 128 * (j + 1)],
                            rhs=ident[:, :],
                            is_transpose=True,
                            start=(qh == 0 and j8 % 4 == 0),
                            stop=(qh == 1 and j8 % 4 == 3),
                        )
                nc.vector.tensor_copy(att_sb[:, :2048], psA[:, :])
                nc.vector.tensor_copy(att_sb[:, 2048:], psB[:, :])

                # ---- kT / qT projections ----
                for b in range(B):
                    for q in range(2):
                        ps = ps_setup.tile([128, 2048], f32, tag="ps")
                        for n in range(4):
                            nc.tensor.matmul(
                                ps[:, 512 * n : 512 * (n + 1)],
                                lhsT=wk4[q][:, :],
                                rhs=hT_sb[:, G * b + 512 * n : G * b + 512 * (n + 1)],
                                start=True,
                                stop=True,
                            )
                        nc.vector.tensor_copy(
                            kT_sb[:, G * (2 * b + q) : G * (2 * b + q + 1)], ps[:, :]
                        )

                ps = ps_setup.tile([128, 2048], f32, tag="ps")
                for b in range(B):
                    for q in range(2):
                        s = 2 * b + q
                        nc.tensor.matmul(
                            ps[:, 256 * s : 256 * (s + 1)],
                            lhsT=wq4[q][:, :],
                            rhs=hqT_sb[:, QS * b : QS * (b + 1)],
                            start=(s % 2 == 0),
                            stop=(s % 2 == 1),
                        )
                nc.vector.tensor_copy(qT_sb[:, :], ps[:, :])

                # ---- v projection into padded 32-slots ----
                nc.vector.memset(v_sb[:, :], 0.0)
                v4 = v_sb[:, :].rearrange("p (b j h s) -> p b j h s", b=B, j=NJ, h=H)
                nc.vector.memset(v4[:, :, :, :, 16:17], 1.0)
                for b in range(B):
                    ps = ps_setup.tile([128, 2048], f32, tag="ps")
                    for j in range(NJ):
                        nc.tensor.matmul(
                            ps[:, 128 * j : 128 * (j + 1)],
                            lhsT=hT_sb[:, G * b + 128 * j : G * b + 128 * (j + 1)],
                            rhs=wv_all[:, :],
                            start=(j % 4 == 0),
                            stop=(j % 4 == 3),
                        )
                    nc.vector.tensor_copy(
                        v4[:, b, :, :, 0:16],
                        ps[:, :].rearrange("p (j h v) -> p j h v", j=NJ, h=H),
                    )

            # setup / attention fence: afterwards no matmul inherits a stale
            # cross-engine wait (S3_LW takes only one)
            tc.strict_bb_all_engine_barrier()

            # ---------------- attention ----------------
            with (
                tc.tile_pool(name="ps_compat", bufs=2, space="PSUM") as ps_compat,
                tc.tile_pool(name="ps_av", bufs=2, space="PSUM") as ps_av,
                tc.tile_pool(name="ps_out", bufs=2, space="PSUM") as ps_out,
            ):
                for b in range(B):
                    for h in range(H):
                        quad, hp = h // 4, h % 4
                        kbase = G * (2 * b + quad)
                        qoff = QS * (2 * b + quad)
                        av = ps_av.tile([128, 256], f32, tag="av")
                        for c4 in range(4):
                            cp = ps_compat.tile([128, 1024], f32, tag="compat")
                            for jj in range(4):
                                j = 4 * c4 + jj
                                nc.tensor.matmul(
                                    cp[:, 256 * jj : 256 * (jj + 1)],
                                    lhsT=kT_sb[
                                        32 * hp : 32 * hp + 16,
                                        kbase + 128 * j : kbase + 128 * (j + 1),
                                    ],
                                    rhs=qT_sb[32 * hp : 32 * hp + 16, qoff : qoff + QS],
                                    start=(jj % 2 == 0),
                                    stop=(jj % 2 == 1),
                                    tile_position=(32 * hp, 0),
                                )
                            e_t = epool.tile([128, 1024], f16, tag="e")
                            nc.scalar.activation(
                                e_t[:, :],
                                cp[:, :],
                                mybir.ActivationFunctionType.Exp,
                                bias=bias_sb[:, :],
                                scale=0.25,
                            )
                            me_t = mepool.tile([128, 1024], f16, tag="me")
                            nc.vector.tensor_mul(
                                me_t[:, :],
                                e_t[:, :],
                                att_sb[:, 1024 * c4 : 1024 * (c4 + 1)],
                            )
                            for jj in range(4):
                                j = 4 * c4 + jj
                                nc.tensor.matmul(
                                    av[32 * jj : 32 * (jj + 1), :],
                                    lhsT=v_sb[
                                        :,
                                        4096 * b + 256 * j + 32 * h : 4096 * b
                                        + 256 * j
                                        + 32 * (h + 1),
                                    ],
                                    rhs=me_t[:, 256 * jj : 256 * (jj + 1)],
                                    start=(c4 == 0),
                                    stop=(c4 == 3),
                                    tile_position=(0, 32 * jj),
                                    skip_group_check=True,
                                )
                        s_t = spool.tile([128, 256], f32, tag="s")
                        nc.vector.tensor_copy(s_t[:, :], av[:, :])
                        for qb in range(2):
                            op = ps_out.tile([128, 17], f32, tag="op")
                            nc.tensor.matmul(
                                op[:, :],
                                lhsT=s_t[:, 128 * qb : 128 * (qb + 1)],
                                rhs=collapse[:, :],
                                start=True,
                                stop=True,
                            )
                            rcp = fpool.tile([128, 1], f32, tag="rcp")
                            nc.vector.reciprocal(rcp[:, :], op[:, 16:17])
                            o_t = fpool.tile([128, 16], f32, tag="o")
                            nc.vector.tensor_scalar(
                                o_t[:, :],
                                op[:, 0:16],
                                rcp[:, :],
                                None,
                                op0=mybir.AluOpType.mult,
                            )
                            dma.dma_start(
                                out=out_dram[b, h, 128 * qb : 128 * (qb + 1), :],
                                in_=o_t[:, :],
                            )

    nc.compile()
    return nc


_NC = None


def _get_nc():
    global _NC
    if _NC is None:
        _NC = build_program()
    return _NC


def make_in_maps(h, mask, W_Q, W_K, W_V):
    h = np.ascontiguousarray(h, dtype=np.float32)
    mask = np.ascontiguousarray(mask, dtype=np.int32)
    W_Q = np.ascontiguousarray(W_Q, dtype=np.float32)
    W_K = np.ascontiguousarray(W_K, dtype=np.float32)
    W_V = np.ascontiguousarray(W_V, dtype=np.float32)
    ident_np = np.eye(128, dtype=np.float16)
    coll_np = np.zeros((128, 17), dtype=np.float32)
    for j in range(4):
        for v in range(17):
            coll_np[32 * j + v, v] = 1.0
    in_maps = []
    for c in range(NCORES):
        sl = slice(QS * c, QS * (c + 1))
        in_maps.append(
            {
                "h": h,
                "hq": np.ascontiguousarray(h[:, sl, :]),
                "maskq": np.ascontiguousarray(mask[sl, :]),
                "W_Q": W_Q,
                "W_K": W_K,
                "W_V": W_V,
                "ident": ident_np,
                "collapse": coll_np,
            }
        )
    return in_maps


def assemble(results):
    full = np.empty((B, H, G, V), dtype=np.float32)
    for c in range(NCORES):
        full[:, :, QS * c : QS * (c + 1), :] = results[c]["out"]
    return full


def kernel(h, mask, W_Q, W_K, W_V, trace=False):
    from concourse.bass_utils import run_bass_kernel_spmd

    nc = _get_nc()
    in_maps = make_in_maps(h, mask, W_Q, W_K, W_V)
    res = run_bass_kernel_spmd(nc, in_maps, core_ids=list(range(NCORES)), trace=trace)
    out = assemble(res.results)
    if trace:
        return out, res
    return out


# revision 9
# speedup vs baseline: 1.0037x; 1.0037x over previous
"""Trainium2 Bass kernel for masked multi-head graph attention (Velickovic).

Problem: h[B=4,G=2048,D=128], mask[G,G] (1=masked), W_Q/W_K[H=8,D,16], W_V[H,D,16]
  q,k,v = h @ W_*  per head; compat = 0.25 * q k^T; masked -> -inf;
  attn = softmax(compat); attn = where(mask, 0, attn); out = attn @ v.

Sharding: 8 cores; core c owns query rows [c*256, (c+1)*256) for ALL (b,h)
pairs. mask rows and h query rows are passed pre-sliced per core; k/v inputs
(full h) are replicated. No cross-device communication.

Per-core algorithm (fp16 on SBUF, fp32 PSUM for matmul accumulation):
  hT = h^T (PE transpose, fp16)      [D=128, G] per b
  kT[h'] = W_K^T hT  (PE, stationary cols padded to 32-row strips)
  qT[h'] = W_Q^T hqT
  v      = hT^T W_V                  [g, (h,v)] 32-wide padded slots:
                                     16 v cols | ones col | 15 zero cols
  attendT = transpose(mask==0) fp16 multiplier  [g, q]
  per (b,h): compatT[g,q] = kT^T qT (PE, 16-contract at row strip 32*h')
             e  = exp(0.25*compatT - 9)   (ACT, PSUM->SBUF fp16; the -9
                  shift cancels in softmax and keeps e within fp16 range)
             me = e * attendT             (DVE fp16 2x)
             S[(strip,v),q] += v_slot^T me  (PE col-tiled, 4 strips)
             out[q,:] = (S^T @ collapse)[:, :16] / (S^T @ collapse)[:, 16]

Hardware constraint honored throughout: a Matmult instruction can carry at
most ONE semaphore wait (walrus S3_LW limit). Every matmul SBUF operand is
produced by the DVE (so operand waits collapse into the single DVE sem),
every PSUM tile's reader in the setup phase is a DVE copy, and a strict
all-engine barrier separates setup from the attention loop so no stale
cross-engine waits surface on matmuls afterwards.
"""

import os
import sys

for p in ("/opt/trn_rl_repo", "/root/.axon_site", "/root/.axon_site/_ro/trn_rl_repo"):
    if os.path.isdir(p) and p not in sys.path:
        sys.path.append(p)

import numpy as np

import concourse.bacc as bacc
import concourse.bass as bass
import concourse.tile as tile
from concourse import mybir

B, G, D, H, K, V = 4, 2048, 128, 8, 16, 16
NCORES = 8
QS = G // NCORES  # 256 q rows per core
NJ = G // 128  # 16 g-blocks
EXP_BIAS = -9.0

f32 = mybir.dt.float32
f16 = mybir.dt.float16
i32 = mybir.dt.int32


def build_program():
    nc = bacc.Bacc("TRN2", target_bir_lowering=False, debug=False)

    h_in = nc.declare_dram_parameter("h", [B, G, D], f32, isOutput=False)
    hq_in = nc.declare_dram_parameter("hq", [B, QS, D], f32, isOutput=False)
    maskq_in = nc.declare_dram_parameter("maskq", [QS, G], i32, isOutput=False)
    wq_in = nc.declare_dram_parameter("W_Q", [H, D, K], f32, isOutput=False)
    wk_in = nc.declare_dram_parameter("W_K", [H, D, K], f32, isOutput=False)
    wv_in = nc.declare_dram_parameter("W_V", [H, D, V], f32, isOutput=False)
    ident_in = nc.declare_dram_parameter("ident", [128, 128], f16, isOutput=False)
    coll_in = nc.declare_dram_parameter("collapse", [128, 17], f32, isOutput=False)
    out_dram = nc.declare_dram_parameter("out", [B, H, QS, V], f32, isOutput=True)

    dma = nc.default_dma_engine

    with tile.TileContext(nc) as tc:
        with (
            tc.tile_pool(name="consts", bufs=1) as consts,
            tc.tile_pool(name="persist", bufs=1) as persist,
            tc.tile_pool(name="stage", bufs=4) as stage,
            tc.tile_pool(name="stagem", bufs=2) as stagem,
            tc.tile_pool(name="epool", bufs=4) as epool,
            tc.tile_pool(name="mepool", bufs=10) as mepool,
            tc.tile_pool(name="spool", bufs=4) as spool,
            tc.tile_pool(name="fpool", bufs=8) as fpool,
        ):
            # ---------------- constants (all DVE-laundered) ----------------
            ident_raw = consts.tile([128, 128], f16, tag="ident_raw")
            dma.dma_start(out=ident_raw[:, :], in_=ident_in[:])
            ident = consts.tile([128, 128], f16, tag="ident")
            nc.vector.tensor_copy(ident[:, :], ident_raw[:, :])

            coll_raw = consts.tile([128, 17], f32, tag="coll_raw")
            dma.dma_start(out=coll_raw[:, :], in_=coll_in[:])
            collapse = consts.tile([128, 17], f32, tag="collapse")
            nc.vector.tensor_copy(collapse[:, :], coll_raw[:, :])

            bias_sb = consts.tile([128, 1], f32, tag="bias")
            nc.vector.memset(bias_sb[:, :], EXP_BIAS)

            # ---------------- persistent SBUF slabs ----------------
            hT_sb = persist.tile([128, B * G], f16, tag="hT")  # [d, b*G+g]
            hqT_sb = persist.tile([128, B * QS], f16, tag="hqT")  # [d, b*QS+q]
            kT_sb = persist.tile([128, B * 2 * G], f16, tag="kT")
            qT_sb = persist.tile([128, B * 2 * QS], f16, tag="qT")
            v_sb = persist.tile([128, B * NJ * H * 32], f16, tag="v")
            att_sb = persist.tile([128, NJ * QS], f16, tag="att")

            wk4 = [
                consts.tile([128, 128], f16, tag=f"wk4_{q}", name=f"wk4_{q}")
                for q in range(2)
            ]
            wq4 = [
                consts.tile([128, 128], f16, tag=f"wq4_{q}", name=f"wq4_{q}")
                for q in range(2)
            ]
            wv_all = consts.tile([128, 128], f16, tag="wv_all")  # [d, 16h+v]

            with tc.tile_pool(name="ps_setup", bufs=2, space="PSUM") as ps_setup:
                # ---- W load + cast (stationary cols 32h'+0:16 = head 4q+h') ----
                for w_in, slabs in ((wk_in, wk4), (wq_in, wq4)):
                    wst = stage.tile([128, 128], f32, tag="wstage")
                    dma.dma_start(
                        out=wst[:, :].rearrange("d (h k) -> d h k", h=H),
                        in_=w_in[:].rearrange("h d k -> d h k"),
                    )
                    for q in range(2):
                        nc.vector.memset(slabs[q][:, :], 0.0)
                        dst = slabs[q][:, :].rearrange("p (s k) -> p s k", k=32)
                        nc.vector.tensor_copy(
                            dst[:, :, 0:16],
                            wst[:, 64 * q : 64 * (q + 1)].rearrange(
                                "p (s k) -> p s k", k=16
                            ),
                        )
                wst = stage.tile([128, 128], f32, tag="wstage")
                dma.dma_start(
                    out=wst[:, :].rearrange("d (h v) -> d h v", h=H),
                    in_=wv_in[:].rearrange("h d v -> d h v"),
                )
                nc.vector.tensor_copy(wv_all[:, :], wst[:, :])

                # ---- hT via fp16 PE transpose ----
                for b in range(B):
                    ps = ps_setup.tile([128, 2048], f16, tag="ps")
                    for j in range(NJ):
                        hst = stage.tile([128, 128], f32, tag="hstage")
                        dma.dma_start(
                            out=hst[:, :], in_=h_in[b, 128 * j : 128 * (j + 1), :]
                        )
                        h16 = stage.tile([128, 128], f16, tag="h16")
                        nc.gpsimd.tensor_copy(h16[:, :], hst[:, :])
                        nc.tensor.matmul(
                            ps[:, 128 * j : 128 * (j + 1)],
                            lhsT=h16[:, :],
                            rhs=ident[:, :],
                            is_transpose=True,
                            start=(j % 8 == 0),
                            stop=(j % 8 == 7),
                        )
                    nc.vector.tensor_copy(hT_sb[:, G * b : G * (b + 1)], ps[:, :])

                ps = ps_setup.tile([128, 1024], f16, tag="ps")
                for b in range(B):
                    for j in range(2):
                        hst = stage.tile([128, 128], f32, tag="hstage")
                        dma.dma_start(
                            out=hst[:, :], in_=hq_in[b, 128 * j : 128 * (j + 1), :]
                        )
                        h16 = stage.tile([128, 128], f16, tag="h16")
                        nc.gpsimd.tensor_copy(h16[:, :], hst[:, :])
                        jj = 2 * b + j
                        nc.tensor.matmul(
                            ps[:, 128 * jj : 128 * (jj + 1)],
                            lhsT=h16[:, :],
                            rhs=ident[:, :],
                            is_transpose=True,
                            start=(jj == 0),
                            stop=(jj == 7),
                        )
                nc.vector.tensor_copy(hqT_sb[:, :], ps[:, :])

                # ---- attendT = transpose(maskq == 0), fp16 ----
                psA = ps_setup.tile([128, 2048], f16, tag="ps", name="psA")
                psB = ps_setup.tile([128, 2048], f16, tag="ps", name="psB")
                for qh in range(2):
                    mi = stagem.tile([128, 2048], i32, tag="mi")
                    dma.dma_start(
                        out=mi[:, :], in_=maskq_in[128 * qh : 128 * (qh + 1), :]
                    )
                    att_f = stagem.tile([128, 2048], f16, tag="attf")
                    nc.vector.tensor_scalar(
                        att_f[:, :], mi[:, :], 0, None, op0=mybir.AluOpType.is_equal
                    )
                    for j in range(NJ):
                        ps = psA if j < 8 else psB
                        j8 = j % 8
                        off = j8 * 256 + qh * 128
                        nc.tensor.matmul(
                            ps[:, off : off + 128],
                            lhsT=att_f[:, 128 * j : 128 * (j + 1)],
                            rhs=ident[:, :],
                            is_transpose=True,
                            start=(qh == 0 and j8 % 4 == 0),
                            stop=(qh == 1 and j8 % 4 == 3),
                        )
                nc.vector.tensor_copy(att_sb[:, :2048], psA[:, :])
                nc.vector.tensor_copy(att_sb[:, 2048:], psB[:, :])

                # ---- kT / qT projections ----
                for b in range(B):
                    for q in range(2):
                        ps = ps_setup.tile([128, 2048], f32, tag="ps")
                        for n in range(4):
                            nc.tensor.matmul(
                                ps[:, 512 * n : 512 * (n + 1)],
                                lhsT=wk4[q][:, :],
                                rhs=hT_sb[:, G * b + 512 * n : G * b + 512 * (n + 1)],
                                start=True,
                                stop=True,
                            )
                        nc.vector.tensor_copy(
                            kT_sb[:, G * (2 * b + q) : G * (2 * b + q + 1)], ps[:, :]
                        )

                ps = ps_setup.tile([128, 2048], f32, tag="ps")
                for b in range(B):
                    for q in range(2):
                        s = 2 * b + q
                        nc.tensor.matmul(
                            ps[:, 256 * s : 256 * (s + 1)],
                            lhsT=wq4[q][:, :],
                            rhs=hqT_sb[:, QS * b : QS * (b + 1)],
                            start=(s % 2 == 0),
                            stop=(s % 2 == 1),
                        )
                nc.vector.tensor_copy(qT_sb[:, :], ps[:, :])

                # ---- v projection into padded 32-slots ----
                nc.vector.memset(v_sb[:, :], 0.0)
                v4 = v_sb[:, :].rearrange("p (b j h s) -> p b j h s", b=B, j=NJ, h=H)
                nc.vector.memset(v4[:, :, :, :, 16:17], 1.0)
                for b in range(B):
                    ps = ps_setup.tile([128, 2048], f32, tag="ps")
                    for j in range(NJ):
                        nc.tensor.matmul(
                            ps[:, 128 * j : 128 * (j + 1)],
                            lhsT=hT_sb[:, G * b + 128 * j : G * b + 128 * (j + 1)],
                            rhs=wv_all[:, :],
                            start=(j % 4 == 0),
                            stop=(j % 4 == 3),
                        )
                    nc.vector.tensor_copy(
                        v4[:, b, :, :, 0:16],
                        ps[:, :].rearrange("p (j h v) -> p j h v", j=NJ, h=H),
                    )

            # setup / attention fence: afterwards no matmul inherits a stale
            # cross-engine wait (S3_LW takes only one)
            tc.strict_bb_all_engine_barrier()

            # ---------------- attention ----------------
            with (
                tc.tile_pool(name="ps_compat", bufs=2, space="PSUM") as ps_compat,
                tc.tile_pool(name="ps_av", bufs=2, space="PSUM") as ps_av,
                tc.tile_pool(name="ps_out", bufs=2, space="PSUM") as ps_out,
            ):
                pairs = [(b, h) for b in range(B) for h in range(H)]
                pending = {}

                def emit_front(pi):
                    b, h = pairs[pi]
                    quad, hp = h // 4, h % 4
                    kbase = G * (2 * b + quad)
                    qoff = QS * (2 * b + quad)
                    mes = []
                    for c4 in range(4):
                        cp = ps_compat.tile(
                            [128, 1024], f32, tag="compat", name=f"cp_{pi}_{c4}"
                        )
                        for jj in range(4):
                            j = 4 * c4 + jj
                            nc.tensor.matmul(
                                cp[:, 256 * jj : 256 * (jj + 1)],
                                lhsT=kT_sb[
                                    32 * hp : 32 * hp + 16,
                                    kbase + 128 * j : kbase + 128 * (j + 1),
                                ],
                                rhs=qT_sb[32 * hp : 32 * hp + 16, qoff : qoff + QS],
                                start=(jj % 2 == 0),
                                stop=(jj % 2 == 1),
                                tile_position=(32 * hp, 0),
                            )
                        e_t = epool.tile([128, 1024], f16, tag="e", name=f"e_{pi}_{c4}")
                        nc.scalar.activation(
                            e_t[:, :],
                            cp[:, :],
                            mybir.ActivationFunctionType.Exp,
                            bias=bias_sb[:, :],
                            scale=0.25,
                        )
                        me_t = mepool.tile(
                            [128, 1024], f16, tag="me", name=f"me_{pi}_{c4}"
                        )
                        nc.vector.tensor_mul(
                            me_t[:, :],
                            e_t[:, :],
                            att_sb[:, 1024 * c4 : 1024 * (c4 + 1)],
                        )
                        mes.append(me_t)
                    pending[pi] = mes

                def emit_tail(pi):
                    b, h = pairs[pi]
                    mes = pending.pop(pi)
                    av = ps_av.tile([128, 256], f32, tag="av", name=f"av_{pi}")
                    for c4 in range(4):
                        me_t = mes[c4]
                        for jj in range(4):
                            j = 4 * c4 + jj
                            nc.tensor.matmul(
                                av[32 * jj : 32 * (jj + 1), :],
                                lhsT=v_sb[
                                    :,
                                    4096 * b + 256 * j + 32 * h : 4096 * b
                                    + 256 * j
                                    + 32 * (h + 1),
                                ],
                                rhs=me_t[:, 256 * jj : 256 * (jj + 1)],
                                start=(c4 == 0),
                                stop=(c4 == 3),
                                tile_position=(0, 32 * jj),
                                skip_group_check=True,
                            )
                    s_t = spool.tile([128, 256], f32, tag="s", name=f"s_{pi}")
                    nc.vector.tensor_copy(s_t[:, :], av[:, :])
                    for qb in range(2):
                        op = ps_out.tile([128, 17], f32, tag="op", name=f"op_{pi}_{qb}")
                        nc.tensor.matmul(
                            op[:, :],
                            lhsT=s_t[:, 128 * qb : 128 * (qb + 1)],
                            rhs=collapse[:, :],
                            start=True,
                            stop=True,
                        )
                        rcp = fpool.tile([128, 1], f32, tag="rcp", name=f"rcp_{pi}_{qb}")
                        nc.vector.reciprocal(rcp[:, :], op[:, 16:17])
                        o_t = fpool.tile([128, 16], f32, tag="o", name=f"o_{pi}_{qb}")
                        nc.vector.tensor_scalar(
                            o_t[:, :],
                            op[:, 0:16],
                            rcp[:, :],
                            None,
                            op0=mybir.AluOpType.mult,
                        )
                        dma.dma_start(
                            out=out_dram[b, h, 128 * qb : 128 * (qb + 1), :],
                            in_=o_t[:, :],
                        )

                for pi in range(len(pairs)):
                    emit_front(pi)
                    if pi > 0:
                        emit_tail(pi - 1)
                emit_tail(len(pairs) - 1)

    nc.compile()
    return nc


_NC = None


def _get_nc():
    global _NC
    if _NC is None:
        _NC = build_program()
    return _NC


def make_in_maps(h, mask, W_Q, W_K, W_V):
    h = np.ascontiguousarray(h, dtype=np.float32)
    mask = np.ascontiguousarray(mask, dtype=np.int32)
    W_Q = np.ascontiguousarray(W_Q, dtype=np.float32)
    W_K = np.ascontiguousarray(W_K, dtype=np.float32)
    W_V = np.ascontiguousarray(W_V, dtype=np.float32)
    ident_np = np.eye(128, dtype=np.float16)
    coll_np = np.zeros((128, 17), dtype=np.float32)
    for j in range(4):
        for v in range(17):
            coll_np[32 * j + v, v] = 1.0
    in_maps = []
    for c in range(NCORES):
        sl = slice(QS * c, QS * (c + 1))
        in_maps.append(
            {
                "h": h,
                "hq": np.ascontiguousarray(h[:, sl, :]),
                "maskq": np.ascontiguousarray(mask[sl, :]),
                "W_Q": W_Q,
                "W_K": W_K,
                "W_V": W_V,
                "ident": ident_np,
                "collapse": coll_np,
            }
        )
    return in_maps


def assemble(results):
    full = np.empty((B, H, G, V), dtype=np.float32)
    for c in range(NCORES):
        full[:, :, QS * c : QS * (c + 1), :] = results[c]["out"]
    return full


def kernel(h, mask, W_Q, W_K, W_V, trace=False):
    from concourse.bass_utils import run_bass_kernel_spmd

    nc = _get_nc()
    in_maps = make_in_maps(h, mask, W_Q, W_K, W_V)
    res = run_bass_kernel_spmd(nc, in_maps, core_ids=list(range(NCORES)), trace=trace)
    out = assemble(res.results)
    if trace:
        return out, res
    return out


# revision 12
# speedup vs baseline: 1.1450x; 1.1408x over previous
"""Trainium2 Bass kernel for masked multi-head graph attention (Velickovic).

Problem: h[B=4,G=2048,D=128], mask[G,G] (1=masked), W_Q/W_K[H=8,D,16], W_V[H,D,16]
  q,k,v = h @ W_*  per head; compat = 0.25 * q k^T; masked -> -inf;
  attn = softmax(compat); attn = where(mask, 0, attn); out = attn @ v.

Sharding: 8 cores; core c owns query rows [c*256, (c+1)*256) for ALL (b,h)
pairs. mask rows and h query rows are passed pre-sliced per core; k/v inputs
(full h) are replicated. No cross-device communication.

Per-core algorithm (fp16 on SBUF, fp32 PSUM for matmul accumulation):
  hT = h^T (PE transpose, fp16)      [D=128, G] per b
  kT[h'] = W_K^T hT  (PE, stationary cols padded to 32-row strips)
  qT[h'] = W_Q^T hqT
  v      = hT^T W_V                  [g, (h,v)] 32-wide padded slots:
                                     16 v cols | ones col | 15 zero cols
  attendT = transpose(mask==0) fp16 multiplier  [g, q]
  per (b,h): compatT[g,q] = kT^T qT (PE, 16-contract at row strip 32*h')
             e  = exp(0.25*compatT - 9)   (ACT, PSUM->SBUF fp16; the -9
                  shift cancels in softmax and keeps e within fp16 range)
             me = e * attendT             (DVE fp16 2x)
             S[(strip,v),q] += v_slot^T me  (PE col-tiled, 4 strips)
             out[q,:] = (S^T @ collapse)[:, :16] / (S^T @ collapse)[:, 16]

Hardware constraint honored throughout: a Matmult instruction can carry at
most ONE semaphore wait (walrus S3_LW limit). Every matmul SBUF operand is
produced by the DVE (so operand waits collapse into the single DVE sem),
every PSUM tile's reader in the setup phase is a DVE copy, and a strict
all-engine barrier separates setup from the attention loop so no stale
cross-engine waits surface on matmuls afterwards.
"""

import os
import sys

for p in ("/opt/trn_rl_repo", "/root/.axon_site", "/root/.axon_site/_ro/trn_rl_repo"):
    if os.path.isdir(p) and p not in sys.path:
        sys.path.append(p)

import numpy as np

import concourse.bacc as bacc
import concourse.bass as bass
import concourse.tile as tile
from concourse import mybir

B, G, D, H, K, V = 4, 2048, 128, 8, 16, 16
NCORES = 8
QS = G // NCORES  # 256 q rows per core
NJ = G // 128  # 16 g-blocks
EXP_BIAS = -9.0

f32 = mybir.dt.float32
f16 = mybir.dt.float16
i32 = mybir.dt.int32


def build_program():
    nc = bacc.Bacc("TRN2", target_bir_lowering=False, debug=False)

    h_in = nc.declare_dram_parameter("h", [B, G, D], f32, isOutput=False)
    hq_in = nc.declare_dram_parameter("hq", [B, QS, D], f32, isOutput=False)
    maskq_in = nc.declare_dram_parameter("maskq", [QS, G], i32, isOutput=False)
    wq_in = nc.declare_dram_parameter("W_Q", [H, D, K], f32, isOutput=False)
    wk_in = nc.declare_dram_parameter("W_K", [H, D, K], f32, isOutput=False)
    wv_in = nc.declare_dram_parameter("W_V", [H, D, V], f32, isOutput=False)
    ident_in = nc.declare_dram_parameter("ident", [128, 128], f16, isOutput=False)
    ident32_in = nc.declare_dram_parameter("ident32", [128, 128], f32, isOutput=False)
    coll_in = nc.declare_dram_parameter("collapse", [128, 17], f32, isOutput=False)
    out_dram = nc.declare_dram_parameter("out", [B, H, QS, V], f32, isOutput=True)

    dma = nc.default_dma_engine

    with tile.TileContext(nc) as tc:
        with (
            tc.tile_pool(name="consts", bufs=1) as consts,
            tc.tile_pool(name="persist", bufs=1) as persist,
            tc.tile_pool(name="stage", bufs=4) as stage,
            tc.tile_pool(name="stagem", bufs=2) as stagem,
            tc.tile_pool(name="epool", bufs=4) as epool,
            tc.tile_pool(name="mepool", bufs=18) as mepool,
            tc.tile_pool(name="spool", bufs=4) as spool,
            tc.tile_pool(name="fpool", bufs=8) as fpool,
        ):
            # ---------------- constants (all DVE-laundered) ----------------
            ident_raw = consts.tile([128, 128], f16, tag="ident_raw")
            dma.dma_start(out=ident_raw[:, :], in_=ident_in[:])
            ident = consts.tile([128, 128], f16, tag="ident")
            nc.vector.tensor_copy(ident[:, :], ident_raw[:, :])
            ident32_raw = consts.tile([128, 128], f32, tag="ident32_raw")
            dma.dma_start(out=ident32_raw[:, :], in_=ident32_in[:])
            ident32 = consts.tile([128, 128], f32, tag="ident32")
            nc.vector.tensor_copy(ident32[:, :], ident32_raw[:, :])

            coll_raw = consts.tile([128, 17], f32, tag="coll_raw")
            dma.dma_start(out=coll_raw[:, :], in_=coll_in[:])
            collapse = consts.tile([128, 17], f32, tag="collapse")
            nc.vector.tensor_copy(collapse[:, :], coll_raw[:, :])

            bias_sb = consts.tile([128, 1], f32, tag="bias")
            nc.vector.memset(bias_sb[:, :], EXP_BIAS)

            # ---------------- persistent SBUF slabs ----------------
            hT_sb = persist.tile([128, B * G], f16, tag="hT")  # [d, b*G+g]
            hqT_sb = persist.tile([128, B * QS], f16, tag="hqT")  # [d, b*QS+q]
            kT_sb = persist.tile([128, B * 2 * G], f16, tag="kT")
            qT_sb = persist.tile([128, B * 2 * QS], f16, tag="qT")
            v_sb = persist.tile([128, B * NJ * H * 32], f16, tag="v")
            att_sb = persist.tile([128, NJ * QS], f16, tag="att")

            wk4 = [
                consts.tile([128, 128], f16, tag=f"wk4_{q}", name=f"wk4_{q}")
                for q in range(2)
            ]
            wq4 = [
                consts.tile([128, 128], f16, tag=f"wq4_{q}", name=f"wq4_{q}")
                for q in range(2)
            ]
            wv_all = consts.tile([128, 128], f16, tag="wv_all")  # [d, 16h+v]

            with tc.tile_pool(name="ps_setup", bufs=2, space="PSUM") as ps_setup:
                # ---- W load + cast (stationary cols 32h'+0:16 = head 4q+h') ----
                for w_in, slabs in ((wk_in, wk4), (wq_in, wq4)):
                    wst = stage.tile([128, 128], f32, tag="wstage")
                    dma.dma_start(
                        out=wst[:, :].rearrange("d (h k) -> d h k", h=H),
                        in_=w_in[:].rearrange("h d k -> d h k"),
                    )
                    for q in range(2):
                        nc.vector.memset(slabs[q][:, :], 0.0)
                        dst = slabs[q][:, :].rearrange("p (s k) -> p s k", k=32)
                        nc.vector.tensor_copy(
                            dst[:, :, 0:16],
                            wst[:, 64 * q : 64 * (q + 1)].rearrange(
                                "p (s k) -> p s k", k=16
                            ),
                        )
                wst = stage.tile([128, 128], f32, tag="wstage")
                dma.dma_start(
                    out=wst[:, :].rearrange("d (h v) -> d h v", h=H),
                    in_=wv_in[:].rearrange("h d v -> d h v"),
                )
                nc.vector.tensor_copy(wv_all[:, :], wst[:, :])

                # ---- hT via fp16 PE transpose ----
                for b in range(B):
                    ps = ps_setup.tile([128, 2048], f32, tag="ps")
                    for j in range(NJ):
                        hst = stage.tile([128, 128], f32, tag="hstage")
                        dma.dma_start(
                            out=hst[:, :], in_=h_in[b, 128 * j : 128 * (j + 1), :]
                        )
                        nc.tensor.matmul(
                            ps[:, 128 * j : 128 * (j + 1)],
                            lhsT=hst[:, :],
                            rhs=ident32[:, :],
                            is_transpose=True,
                            start=(j % 4 == 0),
                            stop=(j % 4 == 3),
                        )
                    nc.vector.tensor_copy(hT_sb[:, G * b : G * (b + 1)], ps[:, :])

                ps = ps_setup.tile([128, 1024], f32, tag="ps")
                for b in range(B):
                    for j in range(2):
                        hst = stage.tile([128, 128], f32, tag="hstage")
                        dma.dma_start(
                            out=hst[:, :], in_=hq_in[b, 128 * j : 128 * (j + 1), :]
                        )
                        jj = 2 * b + j
                        nc.tensor.matmul(
                            ps[:, 128 * jj : 128 * (jj + 1)],
                            lhsT=hst[:, :],
                            rhs=ident32[:, :],
                            is_transpose=True,
                            start=(jj % 4 == 0),
                            stop=(jj % 4 == 3),
                        )
                nc.vector.tensor_copy(hqT_sb[:, :], ps[:, :])

                # ---- attendT = transpose(maskq == 0), fp16 ----
                psA = ps_setup.tile([128, 2048], f16, tag="ps", name="psA")
                psB = ps_setup.tile([128, 2048], f16, tag="ps", name="psB")
                for qh in range(2):
                    mi = stagem.tile([128, 2048], i32, tag="mi")
                    dma.dma_start(
                        out=mi[:, :], in_=maskq_in[128 * qh : 128 * (qh + 1), :]
                    )
                    att_f = stagem.tile([128, 2048], f16, tag="attf")
                    nc.vector.tensor_scalar(
                        att_f[:, :], mi[:, :], 0, None, op0=mybir.AluOpType.is_equal
                    )
                    for j in range(NJ):
                        ps = psA if j < 8 else psB
                        j8 = j % 8
                        off = j8 * 256 + qh * 128
                        nc.tensor.matmul(
                            ps[:, off : off + 128],
                            lhsT=att_f[:, 128 * j : 128 * (j + 1)],
                            rhs=ident[:, :],
                            is_transpose=True,
                            start=(qh == 0 and j8 % 4 == 0),
                            stop=(qh == 1 and j8 % 4 == 3),
                        )
                nc.vector.tensor_copy(att_sb[:, :2048], psA[:, :])
                nc.vector.tensor_copy(att_sb[:, 2048:], psB[:, :])

                # ---- kT / qT projections ----
                for b in range(B):
                    for q in range(2):
                        ps = ps_setup.tile([128, 2048], f32, tag="ps")
                        for n in range(4):
                            nc.tensor.matmul(
                                ps[:, 512 * n : 512 * (n + 1)],
                                lhsT=wk4[q][:, :],
                                rhs=hT_sb[:, G * b + 512 * n : G * b + 512 * (n + 1)],
                                start=True,
                                stop=True,
                            )
                        nc.vector.tensor_copy(
                            kT_sb[:, G * (2 * b + q) : G * (2 * b + q + 1)], ps[:, :]
                        )

                ps = ps_setup.tile([128, 2048], f32, tag="ps")
                for b in range(B):
                    for q in range(2):
                        s = 2 * b + q
                        nc.tensor.matmul(
                            ps[:, 256 * s : 256 * (s + 1)],
                            lhsT=wq4[q][:, :],
                            rhs=hqT_sb[:, QS * b : QS * (b + 1)],
                            start=(s % 2 == 0),
                            stop=(s % 2 == 1),
                        )
                nc.vector.tensor_copy(qT_sb[:, :], ps[:, :])

                # ---- v projection into padded 32-slots ----
                nc.vector.memset(v_sb[:, :], 0.0)
                v4 = v_sb[:, :].rearrange("p (b j h s) -> p b j h s", b=B, j=NJ, h=H)
                nc.vector.memset(v4[:, :, :, :, 16:17], 1.0)
                for b in range(B):
                    ps = ps_setup.tile([128, 2048], f32, tag="ps")
                    for j in range(NJ):
                        nc.tensor.matmul(
                            ps[:, 128 * j : 128 * (j + 1)],
                            lhsT=hT_sb[:, G * b + 128 * j : G * b + 128 * (j + 1)],
                            rhs=wv_all[:, :],
                            start=(j % 4 == 0),
                            stop=(j % 4 == 3),
                        )
                    nc.vector.tensor_copy(
                        v4[:, b, :, :, 0:16],
                        ps[:, :].rearrange("p (j h v) -> p j h v", j=NJ, h=H),
                    )

            # setup / attention fence: afterwards no matmul inherits a stale
            # cross-engine wait (S3_LW takes only one)
            tc.strict_bb_all_engine_barrier()

            # ---------------- attention ----------------
            with (
                tc.tile_pool(name="ps_compat", bufs=2, space="PSUM") as ps_compat,
                tc.tile_pool(name="ps_av", bufs=2, space="PSUM") as ps_av,
                tc.tile_pool(name="ps_out", bufs=2, space="PSUM") as ps_out,
            ):
                pairs = [(b, h) for b in range(B) for h in range(H)]
                NT = len(pairs) // 2  # 2-pair phases
                pending = {}

                def emit_front(t):
                    """Compat+exp+mask for pair 2t ("A") and 2t+1 ("B"),
                    interleaved so adjacent PE matmuls sit on different row
                    strips (hpA != hpB) and execute concurrently."""
                    pA, pB = pairs[2 * t], pairs[2 * t + 1]
                    mes = []
                    for st in range(8):  # chunk-step: g-blocks (2st, 2st+1)
                        cp = ps_compat.tile(
                            [128, 1024], f32, tag="compat", name=f"cp_{t}_{st}"
                        )
                        for m in range(4):
                            half = m % 2  # 0 -> pair A, 1 -> pair B
                            jj = m // 2
                            j = 2 * st + jj
                            b, h = (pA, pB)[half]
                            quad, hp = h // 4, h % 4
                            kbase = G * (2 * b + quad)
                            qoff = QS * (2 * b + quad)
                            # A's blocks land in bank 0 (slices 0,1), B's in
                            # bank 1 (slices 2,3): one accumulation group per
                            # bank
                            sl = 2 * half + jj
                            nc.tensor.matmul(
                                cp[:, 256 * sl : 256 * (sl + 1)],
                                lhsT=kT_sb[
                                    32 * hp : 32 * hp + 16,
                                    kbase + 128 * j : kbase + 128 * (j + 1),
                                ],
                                rhs=qT_sb[32 * hp : 32 * hp + 16, qoff : qoff + QS],
                                start=(jj == 0),
                                stop=(jj == 1),
                                tile_position=(32 * hp, 0),
                            )
                        e_t = epool.tile([128, 1024], f16, tag="e", name=f"e_{t}_{st}")
                        nc.scalar.activation(
                            e_t[:, :],
                            cp[:, :],
                            mybir.ActivationFunctionType.Exp,
                            bias=bias_sb[:, :],
                            scale=0.25,
                        )
                        me_t = mepool.tile(
                            [128, 1024], f16, tag="me", name=f"me_{t}_{st}"
                        )
                        att_sl = att_sb[:, 512 * st : 512 * (st + 1)]
                        att_rep = bass.AP(
                            tensor=att_sl.tensor,
                            offset=att_sl.offset,
                            ap=[att_sl.ap[0], [0, 2]] + list(att_sl.ap[1:]),
                        )
                        nc.vector.tensor_mul(
                            me_t[:, :].rearrange("p (r f) -> p r f", r=2),
                            e_t[:, :].rearrange("p (r f) -> p r f", r=2),
                            att_rep,
                        )
                        mes.append(me_t)
                    pending[t] = mes

                def emit_tail(t):
                    pA, pB = pairs[2 * t], pairs[2 * t + 1]
                    mes = pending.pop(t)
                    av = ps_av.tile([128, 512], f32, tag="av", name=f"av_{t}")
                    # start=True must fire on each strip's FIRST write (it
                    # zeroes that strip's partitions across the whole bank);
                    # stop=True on each strip's last write
                    writes = []
                    for st in range(8):
                        for m in range(4):
                            half, jj = m % 2, m // 2
                            j = 2 * st + jj
                            strip = (j + 2 * half) % 4
                            writes.append((st, half, jj, j, strip))
                    first_of = {}
                    last_of = {}
                    for idx, (st, half, jj, j, strip) in enumerate(writes):
                        first_of.setdefault(strip, idx)
                        last_of[strip] = idx
                    for idx, (st, half, jj, j, strip) in enumerate(writes):
                        me_t = mes[st]
                        b, h = (pA, pB)[half]
                        nc.tensor.matmul(
                            av[
                                32 * strip : 32 * (strip + 1),
                                256 * half : 256 * (half + 1),
                            ],
                            lhsT=v_sb[
                                :,
                                4096 * b + 256 * j + 32 * h : 4096 * b
                                + 256 * j
                                + 32 * (h + 1),
                            ],
                            rhs=me_t[
                                :, 512 * half + 256 * jj : 512 * half + 256 * (jj + 1)
                            ],
                            start=(first_of[strip] == idx),
                            stop=(last_of[strip] == idx),
                            tile_position=(0, 32 * strip),
                            skip_group_check=True,
                        )
                    s_t = spool.tile([128, 512], f32, tag="s", name=f"s_{t}")
                    nc.vector.tensor_copy(s_t[:, :], av[:, :])
                    op = ps_out.tile([128, 4, 17], f32, tag="op", name=f"op_{t}")
                    for m in range(4):
                        half, qb = m // 2, m % 2
                        nc.tensor.matmul(
                            op[:, m, :],
                            lhsT=s_t[:, 256 * half + 128 * qb : 256 * half + 128 * (qb + 1)],
                            rhs=collapse[:, :],
                            start=(m == 0),
                            stop=(m == 3),
                            skip_group_check=True,
                        )
                    rcp = fpool.tile([128, 4], f32, tag="rcp", name=f"rcp_{t}")
                    nc.vector.reciprocal(rcp[:, :], op[:, :, 16])
                    for m in range(4):
                        half, qb = m // 2, m % 2
                        b, h = (pA, pB)[half]
                        o_t = fpool.tile([128, 16], f32, tag="o", name=f"o_{t}_{m}")
                        nc.vector.tensor_scalar(
                            o_t[:, :],
                            op[:, m, 0:16],
                            rcp[:, m : m + 1],
                            None,
                            op0=mybir.AluOpType.mult,
                        )
                        dma.dma_start(
                            out=out_dram[b, h, 128 * qb : 128 * (qb + 1), :],
                            in_=o_t[:, :],
                        )

                for t in range(NT):
                    emit_front(t)
                    if t > 0:
                        emit_tail(t - 1)
                emit_tail(NT - 1)

    nc.compile()
    return nc


_NC = None


def _get_nc():
    global _NC
    if _NC is None:
        _NC = build_program()
    return _NC


def make_in_maps(h, mask, W_Q, W_K, W_V):
    h = np.ascontiguousarray(h, dtype=np.float32)
    mask = np.ascontiguousarray(mask, dtype=np.int32)
    W_Q = np.ascontiguousarray(W_Q, dtype=np.float32)
    W_K = np.ascontiguousarray(W_K, dtype=np.float32)
    W_V = np.ascontiguousarray(W_V, dtype=np.float32)
    ident_np = np.eye(128, dtype=np.float16)
    coll_np = np.zeros((128, 17), dtype=np.float32)
    for j in range(4):
        for v in range(17):
            coll_np[32 * j + v, v] = 1.0
    in_maps = []
    for c in range(NCORES):
        sl = slice(QS * c, QS * (c + 1))
        in_maps.append(
            {
                "h": h,
                "hq": np.ascontiguousarray(h[:, sl, :]),
                "maskq": np.ascontiguousarray(mask[sl, :]),
                "W_Q": W_Q,
                "W_K": W_K,
                "W_V": W_V,
                "ident": ident_np,
                "ident32": np.eye(128, dtype=np.float32),
                "collapse": coll_np,
            }
        )
    return in_maps


def assemble(results):
    full = np.empty((B, H, G, V), dtype=np.float32)
    for c in range(NCORES):
        full[:, :, QS * c : QS * (c + 1), :] = results[c]["out"]
    return full


def kernel(h, mask, W_Q, W_K, W_V, trace=False):
    from concourse.bass_utils import run_bass_kernel_spmd

    nc = _get_nc()
    in_maps = make_in_maps(h, mask, W_Q, W_K, W_V)
    res = run_bass_kernel_spmd(nc, in_maps, core_ids=list(range(NCORES)), trace=trace)
    out = assemble(res.results)
    if trace:
        return out, res
    return out


# revision 17
# speedup vs baseline: 1.1945x; 1.0433x over previous
"""Trainium2 Bass kernel for masked multi-head graph attention (Velickovic).

Problem: h[B=4,G=2048,D=128], mask[G,G] (1=masked), W_Q/W_K[H=8,D,16], W_V[H,D,16]
  q,k,v = h @ W_*  per head; compat = 0.25 * q k^T; masked -> -inf;
  attn = softmax(compat); attn = where(mask, 0, attn); out = attn @ v.

Sharding: 8 cores; core c owns query rows [c*256, (c+1)*256) for ALL (b,h)
pairs. mask rows and h query rows are passed pre-sliced per core; k/v inputs
(full h) are replicated. No cross-device communication.

Per-core algorithm (fp16 on SBUF, fp32 PSUM for matmul accumulation):
  hT = h^T (PE transpose, fp16)      [D=128, G] per b
  kT[h'] = W_K^T hT  (PE, stationary cols padded to 32-row strips)
  qT[h'] = W_Q^T hqT
  v      = hT^T W_V                  [g, (h,v)] 32-wide padded slots:
                                     16 v cols | ones col | 15 zero cols
  attendT = transpose(mask==0) fp16 multiplier  [g, q]
  per (b,h): compatT[g,q] = kT^T qT (PE, 16-contract at row strip 32*h')
             e  = exp(0.25*compatT - 9)   (ACT, PSUM->SBUF fp16; the -9
                  shift cancels in softmax and keeps e within fp16 range)
             me = e * attendT             (DVE fp16 2x)
             S[(strip,v),q] += v_slot^T me  (PE col-tiled, 4 strips)
             out[q,:] = (S^T @ collapse)[:, :16] / (S^T @ collapse)[:, 16]

Hardware constraint honored throughout: a Matmult instruction can carry at
most ONE semaphore wait (walrus S3_LW limit). Every matmul SBUF operand is
produced by the DVE (so operand waits collapse into the single DVE sem),
every PSUM tile's reader in the setup phase is a DVE copy, and a strict
all-engine barrier separates setup from the attention loop so no stale
cross-engine waits surface on matmuls afterwards.
"""

import os
import sys

for p in ("/opt/trn_rl_repo", "/root/.axon_site", "/root/.axon_site/_ro/trn_rl_repo"):
    if os.path.isdir(p) and p not in sys.path:
        sys.path.append(p)

import numpy as np

import concourse.bacc as bacc
import concourse.bass as bass
import concourse.tile as tile
from concourse import mybir

B, G, D, H, K, V = 4, 2048, 128, 8, 16, 16
NCORES = 8
QS = G // NCORES  # 256 q rows per core
NJ = G // 128  # 16 g-blocks
EXP_BIAS = -9.0

f32 = mybir.dt.float32
f16 = mybir.dt.float16
i32 = mybir.dt.int32


def build_program():
    nc = bacc.Bacc("TRN2", target_bir_lowering=False, debug=False)

    h_in = nc.declare_dram_parameter("h", [B, G, D], f32, isOutput=False)
    hq_in = nc.declare_dram_parameter("hq", [B, QS, D], f32, isOutput=False)
    maskq_in = nc.declare_dram_parameter("maskq", [QS, G], i32, isOutput=False)
    wq_in = nc.declare_dram_parameter("W_Q", [H, D, K], f32, isOutput=False)
    wk_in = nc.declare_dram_parameter("W_K", [H, D, K], f32, isOutput=False)
    wv_in = nc.declare_dram_parameter("W_V", [H, D, V], f32, isOutput=False)
    ident_in = nc.declare_dram_parameter("ident", [128, 128], f16, isOutput=False)
    ident32_in = nc.declare_dram_parameter("ident32", [128, 128], f32, isOutput=False)
    coll_in = nc.declare_dram_parameter("collapse", [128, 17], f32, isOutput=False)
    out_dram = nc.declare_dram_parameter("out", [B, H, QS, V], f32, isOutput=True)

    dma = nc.default_dma_engine

    with tile.TileContext(nc) as tc:
        with (
            tc.tile_pool(name="consts", bufs=1) as consts,
            tc.tile_pool(name="persist", bufs=1) as persist,
            tc.tile_pool(name="stage", bufs=4) as stage,
            tc.tile_pool(name="stagem", bufs=2) as stagem,
            tc.tile_pool(name="epool", bufs=6) as epool,
            tc.tile_pool(name="mepool", bufs=18) as mepool,
            tc.tile_pool(name="spool", bufs=4) as spool,
            tc.tile_pool(name="fpool", bufs=8) as fpool,
        ):
            # ---------------- constants (all DVE-laundered) ----------------
            ident_raw = consts.tile([128, 128], f16, tag="ident_raw")
            dma.dma_start(out=ident_raw[:, :], in_=ident_in[:])
            ident = consts.tile([128, 128], f16, tag="ident")
            nc.vector.tensor_copy(ident[:, :], ident_raw[:, :])
            ident32_raw = consts.tile([128, 128], f32, tag="ident32_raw")
            dma.dma_start(out=ident32_raw[:, :], in_=ident32_in[:])
            ident32 = consts.tile([128, 128], f32, tag="ident32")
            nc.vector.tensor_copy(ident32[:, :], ident32_raw[:, :])

            coll_raw = consts.tile([128, 17], f32, tag="coll_raw")
            dma.dma_start(out=coll_raw[:, :], in_=coll_in[:])
            collapse = consts.tile([128, 17], f32, tag="collapse")
            nc.vector.tensor_copy(collapse[:, :], coll_raw[:, :])

            bias_sb = consts.tile([128, 1], f32, tag="bias")
            nc.vector.memset(bias_sb[:, :], EXP_BIAS)

            # ---------------- persistent SBUF slabs ----------------
            hT_sb = persist.tile([128, B * G], f16, tag="hT")  # [d, b*G+g]
            hqT_sb = persist.tile([128, B * QS], f16, tag="hqT")  # [d, b*QS+q]
            kT_sb = persist.tile([128, B * 2 * G], f16, tag="kT")
            qT_sb = persist.tile([128, B * 2 * QS], f16, tag="qT")
            v_sb = persist.tile([128, B * NJ * H * 32], f16, tag="v")
            att_sb = persist.tile([128, NJ * QS], f16, tag="att")

            wk4 = [
                consts.tile([128, 128], f16, tag=f"wk4_{q}", name=f"wk4_{q}")
                for q in range(2)
            ]
            wq4 = [
                consts.tile([128, 128], f16, tag=f"wq4_{q}", name=f"wq4_{q}")
                for q in range(2)
            ]
            wv_all = consts.tile([128, 128], f16, tag="wv_all")  # [d, 16h+v]

            with tc.tile_pool(name="ps_setup", bufs=2, space="PSUM") as ps_setup:
                # ---- W load + cast (stationary cols 32h'+0:16 = head 4q+h') ----
                for w_in, slabs in ((wk_in, wk4), (wq_in, wq4)):
                    wst = stage.tile([128, 128], f32, tag="wstage")
                    dma.dma_start(
                        out=wst[:, :].rearrange("d (h k) -> d h k", h=H),
                        in_=w_in[:].rearrange("h d k -> d h k"),
                    )
                    for q in range(2):
                        nc.vector.memset(slabs[q][:, :], 0.0)
                        dst = slabs[q][:, :].rearrange("p (s k) -> p s k", k=32)
                        nc.vector.tensor_copy(
                            dst[:, :, 0:16],
                            wst[:, 64 * q : 64 * (q + 1)].rearrange(
                                "p (s k) -> p s k", k=16
                            ),
                        )
                wst = stage.tile([128, 128], f32, tag="wstage")
                dma.dma_start(
                    out=wst[:, :].rearrange("d (h v) -> d h v", h=H),
                    in_=wv_in[:].rearrange("h d v -> d h v"),
                )
                nc.vector.tensor_copy(wv_all[:, :], wst[:, :])

                # ---- hT via fp16 PE transpose ----
                for b in range(B):
                    ps = ps_setup.tile([128, 2048], f32, tag="ps")
                    for j in range(NJ):
                        hst = stage.tile([128, 128], f32, tag="hstage")
                        dma.dma_start(
                            out=hst[:, :], in_=h_in[b, 128 * j : 128 * (j + 1), :]
                        )
                        nc.tensor.matmul(
                            ps[:, 128 * j : 128 * (j + 1)],
                            lhsT=hst[:, :],
                            rhs=ident32[:, :],
                            is_transpose=True,
                            start=(j % 4 == 0),
                            stop=(j % 4 == 3),
                        )
                    nc.vector.tensor_copy(hT_sb[:, G * b : G * (b + 1)], ps[:, :])

                ps = ps_setup.tile([128, 1024], f32, tag="ps")
                for b in range(B):
                    for j in range(2):
                        hst = stage.tile([128, 128], f32, tag="hstage")
                        dma.dma_start(
                            out=hst[:, :], in_=hq_in[b, 128 * j : 128 * (j + 1), :]
                        )
                        jj = 2 * b + j
                        nc.tensor.matmul(
                            ps[:, 128 * jj : 128 * (jj + 1)],
                            lhsT=hst[:, :],
                            rhs=ident32[:, :],
                            is_transpose=True,
                            start=(jj % 4 == 0),
                            stop=(jj % 4 == 3),
                        )
                nc.vector.tensor_copy(hqT_sb[:, :], ps[:, :])

                # ---- attendT = transpose(maskq == 0), fp16 ----
                psA = ps_setup.tile([128, 2048], f16, tag="ps", name="psA")
                psB = ps_setup.tile([128, 2048], f16, tag="ps", name="psB")
                for qh in range(2):
                    mi = stagem.tile([128, 2048], i32, tag="mi")
                    dma.dma_start(
                        out=mi[:, :], in_=maskq_in[128 * qh : 128 * (qh + 1), :]
                    )
                    att_f = stagem.tile([128, 2048], f16, tag="attf")
                    nc.vector.tensor_scalar(
                        att_f[:, :], mi[:, :], 0, None, op0=mybir.AluOpType.is_equal
                    )
                    for j in range(NJ):
                        ps = psA if j < 8 else psB
                        j8 = j % 8
                        off = j8 * 256 + qh * 128
                        nc.tensor.matmul(
                            ps[:, off : off + 128],
                            lhsT=att_f[:, 128 * j : 128 * (j + 1)],
                            rhs=ident[:, :],
                            is_transpose=True,
                            start=(qh == 0 and j8 % 4 == 0),
                            stop=(qh == 1 and j8 % 4 == 3),
                        )
                nc.vector.tensor_copy(att_sb[:, :2048], psA[:, :])
                nc.vector.tensor_copy(att_sb[:, 2048:], psB[:, :])

                # ---- kT / qT projections ----
                for b in range(B):
                    for q in range(2):
                        ps = ps_setup.tile([128, 2048], f32, tag="ps")
                        for n in range(4):
                            nc.tensor.matmul(
                                ps[:, 512 * n : 512 * (n + 1)],
                                lhsT=wk4[q][:, :],
                                rhs=hT_sb[:, G * b + 512 * n : G * b + 512 * (n + 1)],
                                start=True,
                                stop=True,
                            )
                        nc.vector.tensor_copy(
                            kT_sb[:, G * (2 * b + q) : G * (2 * b + q + 1)], ps[:, :]
                        )

                ps = ps_setup.tile([128, 2048], f32, tag="ps")
                for b in range(B):
                    for q in range(2):
                        s = 2 * b + q
                        nc.tensor.matmul(
                            ps[:, 256 * s : 256 * (s + 1)],
                            lhsT=wq4[q][:, :],
                            rhs=hqT_sb[:, QS * b : QS * (b + 1)],
                            start=(s % 2 == 0),
                            stop=(s % 2 == 1),
                        )
                nc.vector.tensor_copy(qT_sb[:, :], ps[:, :])

                # ---- v projection into padded 32-slots ----
                nc.vector.memset(v_sb[:, :], 0.0)
                v4 = v_sb[:, :].rearrange("p (b j h s) -> p b j h s", b=B, j=NJ, h=H)
                nc.vector.memset(v4[:, :, :, :, 16:17], 1.0)
                for b in range(B):
                    ps = ps_setup.tile([128, 2048], f32, tag="ps")
                    for j in range(NJ):
                        nc.tensor.matmul(
                            ps[:, 128 * j : 128 * (j + 1)],
                            lhsT=hT_sb[:, G * b + 128 * j : G * b + 128 * (j + 1)],
                            rhs=wv_all[:, :],
                            start=(j % 4 == 0),
                            stop=(j % 4 == 3),
                        )
                    nc.vector.tensor_copy(
                        v4[:, b, :, :, 0:16],
                        ps[:, :].rearrange("p (j h v) -> p j h v", j=NJ, h=H),
                    )

            # setup / attention fence: afterwards no matmul inherits a stale
            # cross-engine wait (S3_LW takes only one)
            tc.strict_bb_all_engine_barrier()

            # ---------------- attention ----------------
            with (
                tc.tile_pool(name="ps_compat", bufs=3, space="PSUM") as ps_compat,
                tc.tile_pool(name="ps_av", bufs=1, space="PSUM") as ps_av,
                tc.tile_pool(name="ps_out", bufs=1, space="PSUM") as ps_out,
            ):
                pairs = [(b, h) for b in range(B) for h in range(H)]
                NT = len(pairs) // 2  # 2-pair phases
                pending = {}

                def emit_front(t):
                    """Compat+exp+mask for pair 2t ("A") and 2t+1 ("B"),
                    interleaved so adjacent PE matmuls sit on different row
                    strips (hpA != hpB) and execute concurrently."""
                    pA, pB = pairs[2 * t], pairs[2 * t + 1]
                    mes = []
                    for st in range(8):  # chunk-step: g-blocks (2st, 2st+1)
                        cp = ps_compat.tile(
                            [128, 1024], f32, tag="compat", name=f"cp_{t}_{st}"
                        )
                        for m in range(4):
                            half = m % 2  # 0 -> pair A, 1 -> pair B
                            jj = m // 2
                            j = 2 * st + jj
                            b, h = (pA, pB)[half]
                            quad, hp = h // 4, h % 4
                            kbase = G * (2 * b + quad)
                            qoff = QS * (2 * b + quad)
                            sl = 2 * half + jj
                            nc.tensor.matmul(
                                cp[:, 256 * sl : 256 * (sl + 1)],
                                lhsT=kT_sb[
                                    32 * hp : 32 * hp + 16,
                                    kbase + 128 * j : kbase + 128 * (j + 1),
                                ],
                                rhs=qT_sb[32 * hp : 32 * hp + 16, qoff : qoff + QS],
                                start=(jj == 0),
                                stop=(jj == 1),
                                tile_position=(32 * hp, 0),
                            )
                        e_t = epool.tile([128, 1024], f16, tag="e", name=f"e_{t}_{st}")
                        nc.scalar.activation(
                            e_t[:, :],
                            cp[:, :],
                            mybir.ActivationFunctionType.Exp,
                            bias=bias_sb[:, :],
                            scale=0.25,
                        )
                        me_t = mepool.tile(
                            [128, 1024], f16, tag="me", name=f"me_{t}_{st}"
                        )
                        att_sl = att_sb[:, 512 * st : 512 * (st + 1)]
                        att_rep = bass.AP(
                            tensor=att_sl.tensor,
                            offset=att_sl.offset,
                            ap=[att_sl.ap[0], [0, 2]] + list(att_sl.ap[1:]),
                        )
                        nc.vector.tensor_mul(
                            me_t[:, :].rearrange("p (r f) -> p r f", r=2),
                            e_t[:, :].rearrange("p (r f) -> p r f", r=2),
                            att_rep,
                        )
                        mes.append(me_t)
                    pending[t] = mes

                def emit_tail(t):
                    pA, pB = pairs[2 * t], pairs[2 * t + 1]
                    mes = pending.pop(t)
                    av = ps_av.tile([128, 512], f32, tag="av", name=f"av_{t}")
                    writes = []
                    for st in range(8):
                        for m in range(4):
                            half, jj = m % 2, m // 2
                            j = 2 * st + jj
                            strip = (j + 2 * half) % 4
                            writes.append((st, half, jj, j, strip))
                    first_of = {}
                    last_of = {}
                    for idx, (st, half, jj, j, strip) in enumerate(writes):
                        first_of.setdefault(strip, idx)
                        last_of[strip] = idx
                    for idx, (st, half, jj, j, strip) in enumerate(writes):
                        me_t = mes[st]
                        b, h = (pA, pB)[half]
                        nc.tensor.matmul(
                            av[
                                32 * strip : 32 * (strip + 1),
                                256 * half : 256 * (half + 1),
                            ],
                            lhsT=v_sb[
                                :,
                                4096 * b + 256 * j + 32 * h : 4096 * b
                                + 256 * j
                                + 32 * (h + 1),
                            ],
                            rhs=me_t[
                                :, 512 * half + 256 * jj : 512 * half + 256 * (jj + 1)
                            ],
                            start=(first_of[strip] == idx),
                            stop=(last_of[strip] == idx),
                            tile_position=(0, 32 * strip),
                            skip_group_check=True,
                        )
                    s_t = spool.tile([128, 512], f32, tag="s", name=f"s_{t}")
                    nc.vector.tensor_copy(s_t[:, :], av[:, :])
                    op = ps_out.tile([128, 4, 17], f32, tag="op", name=f"op_{t}")
                    for m in range(4):
                        half, qb = m // 2, m % 2
                        nc.tensor.matmul(
                            op[:, m, :],
                            lhsT=s_t[:, 256 * half + 128 * qb : 256 * half + 128 * (qb + 1)],
                            rhs=collapse[:, :],
                            start=(m == 0),
                            stop=(m == 3),
                            skip_group_check=True,
                        )
                    rcp = fpool.tile([128, 4], f32, tag="rcp", name=f"rcp_{t}")
                    nc.vector.reciprocal(rcp[:, :], op[:, :, 16])
                    for m in range(4):
                        half, qb = m // 2, m % 2
                        b, h = (pA, pB)[half]
                        o_t = fpool.tile([128, 16], f32, tag="o", name=f"o_{t}_{m}")
                        nc.vector.tensor_scalar(
                            o_t[:, :],
                            op[:, m, 0:16],
                            rcp[:, m : m + 1],
                            None,
                            op0=mybir.AluOpType.mult,
                        )
                        dma.dma_start(
                            out=out_dram[b, h, 128 * qb : 128 * (qb + 1), :],
                            in_=o_t[:, :],
                        )

                for t in range(NT):
                    emit_front(t)
                    if t > 0:
                        emit_tail(t - 1)
                emit_tail(NT - 1)

    nc.compile()
    return nc


_NC = None


def _get_nc():
    global _NC
    if _NC is None:
        _NC = build_program()
    return _NC


def make_in_maps(h, mask, W_Q, W_K, W_V):
    h = np.ascontiguousarray(h, dtype=np.float32)
    mask = np.ascontiguousarray(mask, dtype=np.int32)
    W_Q = np.ascontiguousarray(W_Q, dtype=np.float32)
    W_K = np.ascontiguousarray(W_K, dtype=np.float32)
    W_V = np.ascontiguousarray(W_V, dtype=np.float32)
    ident_np = np.eye(128, dtype=np.float16)
    coll_np = np.zeros((128, 17), dtype=np.float32)
    for j in range(4):
        for v in range(17):
            coll_np[32 * j + v, v] = 1.0
    in_maps = []
    for c in range(NCORES):
        sl = slice(QS * c, QS * (c + 1))
        in_maps.append(
            {
                "h": h,
                "hq": np.ascontiguousarray(h[:, sl, :]),
                "maskq": np.ascontiguousarray(mask[sl, :]),
                "W_Q": W_Q,
                "W_K": W_K,
                "W_V": W_V,
                "ident": ident_np,
                "ident32": np.eye(128, dtype=np.float32),
                "collapse": coll_np,
            }
        )
    return in_maps


def assemble(results):
    full = np.empty((B, H, G, V), dtype=np.float32)
    for c in range(NCORES):
        full[:, :, QS * c : QS * (c + 1), :] = results[c]["out"]
    return full


def kernel(h, mask, W_Q, W_K, W_V, trace=False):
    from concourse.bass_utils import run_bass_kernel_spmd

    nc = _get_nc()
    in_maps = make_in_maps(h, mask, W_Q, W_K, W_V)
    res = run_bass_kernel_spmd(nc, in_maps, core_ids=list(range(NCORES)), trace=trace)
    out = assemble(res.results)
    if trace:
        return out, res
    return out
